# revision 3
# baseline (speedup 1.0000x reference)
"""ConvLSTM2D iterative kernel for Trainium2 (Bass/Tile), 8-core batch-parallel.

v3.2 design (per core, B_local=2 of B=16), all fp16 on-chip:
  - h kept channel-major in one padded image tile H4 [128, 2*HPAD]:
    cols 0:HPAD   "h2": L = h shifted +RS (one row down), U = h canonical
    cols HPAD:2H  "h3": L = h shifted +1 (one px right), U = h canonical
  - 3x3 conv = 5 PSUM-accumulated matmuls per (chunk, nh): tap-pair groups
    g0..g2 (h2), g3 (h3), and g4 = h2-view at +1 col with zero upper
    weights (solo tap (+1,+1) rides the lower half).
  - Gate chunk layout: ck0 = [o ; i], ck1 = [f ; c]; psum tile [128,2048]
    (cols 0:1024 ck0, 1024:2048 ck1). Per-chunk bias vectors (BI tile).
  - Per-tile gate chain, pipelined 3 blocks deep (block = one pixel tile):
      block b:   relu1, tanhC, relu0 (ACT, frees psum) ; min1 (DVE 4x) ;
                 of-DMA: of@U <- min1'd [o|f]@L (one cross DMA, off-chain) ;
                 u = i'*tC (DVE, all upper)
      block b+1: v: C *= f'  ;  cnew: C += u   (DVE, upper)
      block b+2: tanh2 (ACT) ; h-write: H4-h2@U interior = o'*tc2 (DVE TT,
                 strided) ; then per 2 tiles: h2@L/h3@L cross DMAs (SP),
                 h3@U same-partition copy (Pool), HBM out (fp16).
  - Two samples per core are software-pipelined in half-steps (sample k%2,
    step k//2), so the PE never waits at step boundaries; all DMA deps are
    ready several microseconds before the SP sequencer reaches them.
"""

import numpy as np

import concourse.mybir as mybir
from concourse import bacc
from concourse.tile import TileContext
from concourse.bass_utils import run_bass_kernel_spmd

F32 = mybir.dt.float32
F16 = mybir.dt.float16
AF = mybir.ActivationFunctionType
ALU = mybir.AluOpType

NCORES = 8
B, T, H, W, F = 16, 16, 64, 64, 64
BLOC = B // NCORES          # 2 samples per core
RS = W + 2                  # padded row stride
SROWS = H + 2
SBLK = RS * SROWS           # padded elems per sample
GUARD = RS
HPAD = GUARD + BLOC * SBLK + 2
PIX = BLOC * H * W          # 8192 px per core
NPT = 8                     # pixel tiles of 1024 (16 rows)
TPS = NPT // BLOC           # tiles per sample (4)
ROWS_PT = 16
NBLK = 2 * T * TPS          # global pipeline blocks

# tap groups: (tapA upper, tapB lower, view) g0..g3; g4 solo (see wblob)
GROUPS = [
    ((0, 0), (1, 0), 0),     # view 0 = h2 (cols 0:HPAD)
    ((0, 1), (1, 1), 0),
    ((0, 2), (1, 2), 0),
    ((2, 0), (2, 1), 1),     # view 1 = h3 (cols HPAD:2*HPAD)
]
GROUP_OFF = [-RS - 1, -RS, -RS + 1, RS - 1, 1]   # g4: h2-view at +1 col

# gate order within psum chunks: ck0 = [o;i], ck1 = [f;c]
CK_GATES = [(192, 0), (64, 128)]   # (lower-half gate base, upper-half base)

LAST_EXEC_NS = None
LAST_RESULTS = None


def _ppos(s, y, x):
    return GUARD + s * SBLK + (y + 1) * RS + x


def build_program():
    nc = bacc.Bacc("TRN2", target_bir_lowering=False, debug=False,
                   num_devices=NCORES)

    h4_d = nc.dram_tensor("h4_init", [128, 2 * HPAD], F16, kind="ExternalInput").ap()
    c_d = nc.dram_tensor("c_init", [64, PIX], F16, kind="ExternalInput").ap()
    w_d = nc.dram_tensor("wblob", [T, 128, 1280], F16, kind="ExternalInput").ap()
    bi_d = nc.dram_tensor("biases", [128, 3 * T], F32, kind="ExternalInput").ap()
    o_d = nc.dram_tensor("out", [T, F, PIX], F16, kind="ExternalOutput").ap()

    with TileContext(nc) as tc:
        with (
            tc.tile_pool(name="state", bufs=1) as spool,
            tc.tile_pool(name="wp", bufs=2) as wpool,
            tc.tile_pool(name="psA", bufs=2, space="PSUM") as psApool,
            tc.tile_pool(name="psB", bufs=2, space="PSUM") as psBpool,
            tc.tile_pool(name="g01p", bufs=4) as g01pool,
            tc.tile_pool(name="ofp", bufs=6) as ofpool,
            tc.tile_pool(name="tcxp", bufs=5) as tcxpool,
            tc.tile_pool(name="tc2p", bufs=5) as tc2pool,
            tc.tile_pool(name="up", bufs=4) as upool,
        ):
            H4 = spool.tile([128, 2 * HPAD], F16)
            C = spool.tile([128, PIX], F16)
            BI = spool.tile([128, 3 * T], F32)
            wsb = [None] * T
            wsb[0] = wpool.tile([128, 1280], F16, name="wsb0")
            # order: first tiles' data (h2/h3 of sample 0) before the rest
            nc.sync.dma_start(wsb[0][:, :], w_d[0, :, :])
            Q = HPAD // 4   # 8 chunks of Q cover [0, 2*HPAD)
            for q in (0, 4, 1, 5):
                sl = slice(q * Q, (q + 1) * Q)
                nc.sync.dma_start(H4[:, sl], h4_d[:, sl])
            nc.sync.dma_start(BI[:, :], bi_d[:, :])
            for q in (2, 6, 3, 7):
                sl = slice(q * Q, (q + 1) * Q)
                nc.sync.dma_start(H4[:, sl], h4_d[:, sl])
            nc.sync.dma_start(C[64:128, :], c_d[:, :])

            def emit_mms(t, s, j):
                """matmuls + psum activations for tile j (0..3) of sample s."""
                psA = psApool.tile([128, 1024], F32)
                psB = psBpool.tile([128, 1024], F32)
                for ck in (1, 0):
                    ps = psB if ck == 1 else psA
                    for nh in range(2):
                        row0 = j * ROWS_PT + nh * 8
                        p0 = _ppos(s, row0, 0)
                        for g in range(5):
                            view = GROUPS[g][2] if g < 4 else 0
                            base = view * HPAD + p0 + GROUP_OFF[g]
                            rhs = (H4[:, base:base + 8 * RS]
                                   .rearrange("p (r c) -> p r c", r=8, c=RS)
                                   [:, :, 0:W])
                            lhsT = wsb[t][:, (g * 2 + ck) * 128:
                                          (g * 2 + ck) * 128 + 128]
                            nc.tensor.matmul(
                                ps[:, nh * 512:nh * 512 + 512],
                                lhsT, rhs,
                                start=(g == 0), stop=(g == 4),
                            )
                g01 = g01pool.tile([128, 2048], F16)
                tcx = tcxpool.tile([128, 1024], F16)
                nc.scalar.activation(g01[:, 1024:2048], psB[:, :],
                                     AF.Relu, bias=BI[:, 3 * t + 1:3 * t + 2],
                                     scale=0.2)
                nc.scalar.activation(tcx[64:128, :], psB[64:128, :],
                                     AF.Tanh,
                                     bias=BI[64:128, 3 * t + 2:3 * t + 3])
                nc.scalar.activation(g01[:, 0:1024], psA[:, :],
                                     AF.Relu, bias=BI[:, 3 * t:3 * t + 1],
                                     scale=0.2)
                return g01, tcx

            def emit_min_u(g01, tcx):
                """clip + of-cross + u = i'*tC (upper half)."""
                of = ofpool.tile([128, 2048], F16)
                u = upool.tile([128, 1024], F16)
                nc.vector.tensor_scalar_min(g01[:, :], g01[:, :], 1.0)
                nc.sync.dma_start(of[64:128, :], g01[0:64, :])
                nc.vector.tensor_tensor(u[64:128, :], g01[64:128, 0:1024],
                                        tcx[64:128, :], op=ALU.mult)
                return of, u

            def emit_vc(pt, of, u):
                """c-state update for tile pt (all upper half)."""
                tsl = slice(pt * 1024, (pt + 1) * 1024)
                nc.vector.tensor_tensor(C[64:128, tsl], of[64:128, 1024:2048],
                                        C[64:128, tsl], op=ALU.mult)
                nc.vector.tensor_tensor(C[64:128, tsl], C[64:128, tsl],
                                        u[64:128, :], op=ALU.add)

            def emit_finish(s, pt, of):
                """tanh2 + h-write into H4-h2@U interior for tile pt."""
                tsl = slice(pt * 1024, (pt + 1) * 1024)
                rg = pt % TPS
                tc2 = tc2pool.tile([128, 1024], F16)
                nc.scalar.activation(tc2[64:128, :], C[64:128, tsl], AF.Tanh)
                a0 = _ppos(s, rg * ROWS_PT, 0)
                hdst = (H4[64:128, a0:a0 + ROWS_PT * RS]
                        .rearrange("p (r c) -> p r c", r=ROWS_PT, c=RS)
                        [:, :, 0:W])
                nc.vector.tensor_tensor(
                    hdst,
                    of[64:128, 0:1024].rearrange("p (r c) -> p r c",
                                                 r=ROWS_PT, c=W),
                    tc2[64:128, :].rearrange("p (r c) -> p r c",
                                             r=ROWS_PT, c=W),
                    op=ALU.mult)

            def emit_pair_dmas(t, s, jhi):
                """shifted copies + HBM out for tiles (jhi-1, jhi)."""
                pt = s * TPS + jhi
                lo = GUARD + s * SBLK + ((jhi - 1) * ROWS_PT + 1) * RS
                hi = lo + 2 * ROWS_PT * RS
                if t < T - 1:
                    # cross-partition shifted lowers: h2@L (-RS), h3@L (-1)
                    nc.sync.dma_start(H4[0:64, lo - RS:hi - RS],
                                      H4[64:128, lo:hi])
                    nc.sync.dma_start(H4[0:64, HPAD + lo - 1:HPAD + hi - 1],
                                      H4[64:128, lo:hi])
                    # same-partition canonical copy: h3@U (Pool engine)
                    nc.gpsimd.tensor_copy(H4[64:128, HPAD + lo:HPAD + hi],
                                          H4[64:128, lo:hi])
                a0p = _ppos(s, (jhi - 1) * ROWS_PT, 0)
                src = (H4[64:128, a0p:a0p + 2 * ROWS_PT * RS]
                       .rearrange("p (r c) -> p r c", r=2 * ROWS_PT, c=RS)
                       [:, :, 0:W])
                osl = slice((pt - 1) * 1024, (pt + 1) * 1024)
                nc.sync.dma_start(
                    o_d[t, :, osl].rearrange("f (r c) -> f r c",
                                             r=2 * ROWS_PT, c=W),
                    src)

            # global 3-deep block pipeline over half-steps
            live = {}   # block index -> stage state
            def block_ts(b):
                k, j = divmod(b, TPS)
                return k // 2, k % 2, j

            for b in range(NBLK + 2):
                if b < NBLK:
                    t, s, j = block_ts(b)
                    if j == 0 and s == 0 and t + 1 < T:
                        wsb[t + 1] = wpool.tile([128, 1280], F16,
                                                name=f"wsb{t + 1}")
                        nc.sync.dma_start(wsb[t + 1][:, :], w_d[t + 1, :, :])
                    g01, tcx = emit_mms(t, s, j)
                    of, u = emit_min_u(g01, tcx)
                    live[b] = (of, u)
                if b >= 1 and b - 1 < NBLK:
                    t1, s1, j1 = block_ts(b - 1)
                    of1, u1 = live[b - 1]
                    emit_vc(s1 * TPS + j1, of1, u1)
                if b >= 2:
                    tf, sf, jf = block_ts(b - 2)
                    off, _ = live.pop(b - 2)
                    emit_finish(sf, sf * TPS + jf, off)
                    if jf % 2 == 1:
                        emit_pair_dmas(tf, sf, jf)

    nc.compile()
    return nc


_CACHED_NC = None


def _get_nc():
    global _CACHED_NC
    if _CACHED_NC is None:
        _CACHED_NC = build_program()
    return _CACHED_NC


def _host_pack(inputs, h0, c0, kernels, rec_kernels, bias):
    f16 = mybir.dt.np(F16)
    Wf = (kernels + rec_kernels).astype(np.float32)  # [T,3,3,64,256]
    wblob = np.zeros((T, 128, 1280), np.float32)
    for g in range(4):
        ta, tb, _ = GROUPS[g]
        for ck, (glo, gup) in enumerate(CK_GATES):
            col = (g * 2 + ck) * 128
            wblob[:, 64:128, col:col + 64] = Wf[:, ta[0], ta[1], :, glo:glo + 64]
            wblob[:, 64:128, col + 64:col + 128] = Wf[:, ta[0], ta[1], :, gup:gup + 64]
            wblob[:, 0:64, col:col + 64] = Wf[:, tb[0], tb[1], :, glo:glo + 64]
            wblob[:, 0:64, col + 64:col + 128] = Wf[:, tb[0], tb[1], :, gup:gup + 64]
    for ck, (glo, gup) in enumerate(CK_GATES):
        col = (4 * 2 + ck) * 128
        wblob[:, 0:64, col:col + 64] = Wf[:, 2, 2, :, glo:glo + 64]
        wblob[:, 0:64, col + 64:col + 128] = Wf[:, 2, 2, :, gup:gup + 64]
    wblob = np.ascontiguousarray(wblob.astype(f16))

    bz = bias.astype(np.float32)  # [T, 256]
    biases = np.zeros((128, 3 * T), np.float32)
    for t in range(T):
        biases[0:64, 3 * t] = 0.2 * bz[t, 192:256] + 0.5      # o (ck0 @L)
        biases[64:128, 3 * t] = 0.2 * bz[t, 0:64] + 0.5       # i (ck0 @U)
        biases[0:64, 3 * t + 1] = 0.2 * bz[t, 64:128] + 0.5   # f (ck1 @L)
        biases[64:128, 3 * t + 2] = bz[t, 128:192]            # c tanh bias
    biases = np.ascontiguousarray(biases)

    in_maps = []
    for core in range(NCORES):
        b0 = core * BLOC
        hp = np.zeros((64, HPAD), np.float32)
        cpx = np.zeros((64, PIX), np.float32)
        for s in range(BLOC):
            hsamp = np.transpose(h0[b0 + s], (2, 0, 1)).reshape(64, H * W)
            csamp = np.transpose(c0[b0 + s], (2, 0, 1)).reshape(64, H * W)
            for y in range(H):
                p = _ppos(s, y, 0)
                hp[:, p:p + W] = hsamp[:, y * W:(y + 1) * W]
            cpx[:, s * H * W:(s + 1) * H * W] = csamp
        h2i = np.zeros((128, HPAD), np.float32)
        h3i = np.zeros((128, HPAD), np.float32)
        h2i[64:128] = hp
        h3i[64:128] = hp
        h2i[0:64, :HPAD - RS] = hp[:, RS:]
        h3i[0:64, :HPAD - 1] = hp[:, 1:]
        h4 = np.concatenate([h2i, h3i], axis=1)
        in_maps.append({
            "h4_init": np.ascontiguousarray(h4.astype(f16)),
            "c_init": np.ascontiguousarray(cpx.astype(f16)),
            "wblob": wblob,
            "biases": biases,
        })
    return in_maps


def kernel(inputs, h0, c0, kernels, rec_kernels, bias):
    global LAST_EXEC_NS, LAST_RESULTS
    h0 = np.asarray(h0, np.float32)
    c0 = np.asarray(c0, np.float32)
    kernels = np.asarray(kernels, np.float32)
    rec_kernels = np.asarray(rec_kernels, np.float32)
    bias = np.asarray(bias, np.float32)

    nc = _get_nc()
    in_maps = _host_pack(inputs, h0, c0, kernels, rec_kernels, bias)
    import os
    trace = bool(int(os.environ.get("K_TRACE", "0")))
    res = run_bass_kernel_spmd(nc, in_maps, core_ids=list(range(NCORES)),
                               trace=trace)
    LAST_RESULTS = res
    LAST_EXEC_NS = res.exec_time_ns
    if LAST_EXEC_NS is None:
        try:
            from concourse.timeline_sim import TimelineSim
            LAST_EXEC_NS = int(TimelineSim(nc, no_exec=True).simulate())
        except Exception:
            pass

    out = np.empty((B, T, H, W, F), np.float32)
    for core in range(NCORES):
        o = res.results[core]["out"].astype(np.float32).reshape(T, F, BLOC, H, W)
        out[core * BLOC:(core + 1) * BLOC] = np.transpose(o, (2, 0, 3, 4, 1))
    return out


# revision 4
# speedup vs baseline: 1.0025x; 1.0025x over previous
"""ConvLSTM2D iterative kernel for Trainium2 (Bass/Tile), 8-core batch-parallel.

v3.2 design (per core, B_local=2 of B=16), all fp16 on-chip:
  - h kept channel-major in one padded image tile H4 [128, 2*HPAD]:
    cols 0:HPAD   "h2": L = h shifted +RS (one row down), U = h canonical
    cols HPAD:2H  "h3": L = h shifted +1 (one px right), U = h canonical
  - 3x3 conv = 5 PSUM-accumulated matmuls per (chunk, nh): tap-pair groups
    g0..g2 (h2), g3 (h3), and g4 = h2-view at +1 col with zero upper
    weights (solo tap (+1,+1) rides the lower half).
  - Gate chunk layout: ck0 = [o ; i], ck1 = [f ; c]; two psum pools of
    [128,1024] tiles (2 slots each) recycle independently per chunk.
    Per-chunk bias vectors (BI tile).
  - Per-tile gate chain, pipelined 3 blocks deep (block = one pixel tile):
      block b:   relu1, tanhC, relu0 (ACT, frees psum) ; min1 (DVE 4x) ;
                 of-DMA: of@U <- min1'd [o|f]@L (one cross DMA, off-chain) ;
                 u = i'*tC (DVE, all upper)
      block b+1: v: C *= f'  ;  cnew: C += u   (DVE, upper)
      block b+2: tanh2 (ACT) ; h-write: H4-h2@U interior = o'*tc2 (DVE TT,
                 strided) ; then per 2 tiles: h2@L/h3@L cross DMAs (SP),
                 h3@U same-partition copy (Pool), HBM out (fp16).
  - Two samples per core are software-pipelined in half-steps (sample k%2,
    step k//2), so the PE never waits at step boundaries; all DMA deps are
    ready several microseconds before the SP sequencer reaches them.
"""

import numpy as np

import concourse.mybir as mybir
from concourse import bacc
from concourse.tile import TileContext
from concourse.bass_utils import run_bass_kernel_spmd

F32 = mybir.dt.float32
F16 = mybir.dt.float16
AF = mybir.ActivationFunctionType
ALU = mybir.AluOpType

NCORES = 8
B, T, H, W, F = 16, 16, 64, 64, 64
BLOC = B // NCORES          # 2 samples per core
RS = W + 2                  # padded row stride
SROWS = H + 2
SBLK = RS * SROWS           # padded elems per sample
GUARD = RS
HPAD = GUARD + BLOC * SBLK + 2
PIX = BLOC * H * W          # 8192 px per core
NPT = 8                     # pixel tiles of 1024 (16 rows)
TPS = NPT // BLOC           # tiles per sample (4)
ROWS_PT = 16
NBLK = 2 * T * TPS          # global pipeline blocks

# tap groups: (tapA upper, tapB lower, view) g0..g3; g4 solo (see wblob)
GROUPS = [
    ((0, 0), (1, 0), 0),     # view 0 = h2 (cols 0:HPAD)
    ((0, 1), (1, 1), 0),
    ((0, 2), (1, 2), 0),
    ((2, 0), (2, 1), 1),     # view 1 = h3 (cols HPAD:2*HPAD)
]
GROUP_OFF = [-RS - 1, -RS, -RS + 1, RS - 1, 1]   # g4: h2-view at +1 col

# gate order within psum chunks: ck0 = [o;i], ck1 = [f;c]
CK_GATES = [(192, 0), (64, 128)]   # (lower-half gate base, upper-half base)

LAST_EXEC_NS = None
LAST_RESULTS = None


def _ppos(s, y, x):
    return GUARD + s * SBLK + (y + 1) * RS + x


def build_program():
    nc = bacc.Bacc("TRN2", target_bir_lowering=False, debug=False,
                   num_devices=NCORES)

    h4_d = nc.dram_tensor("h4_init", [128, 2 * HPAD], F16, kind="ExternalInput").ap()
    c_d = nc.dram_tensor("c_init", [64, PIX], F16, kind="ExternalInput").ap()
    w_d = nc.dram_tensor("wblob", [T, 128, 1280], F16, kind="ExternalInput").ap()
    bi_d = nc.dram_tensor("biases", [128, 3 * T], F32, kind="ExternalInput").ap()
    o_d = nc.dram_tensor("out", [T, F, PIX], F16, kind="ExternalOutput").ap()

    with TileContext(nc) as tc:
        with (
            tc.tile_pool(name="state", bufs=1) as spool,
            tc.tile_pool(name="wp", bufs=2) as wpool,
            tc.tile_pool(name="psA", bufs=2, space="PSUM") as psApool,
            tc.tile_pool(name="psB", bufs=2, space="PSUM") as psBpool,
            tc.tile_pool(name="g01p", bufs=4) as g01pool,
            tc.tile_pool(name="ofp", bufs=6) as ofpool,
            tc.tile_pool(name="tcxp", bufs=5) as tcxpool,
            tc.tile_pool(name="tc2p", bufs=5) as tc2pool,
            tc.tile_pool(name="up", bufs=4) as upool,
        ):
            H4 = spool.tile([128, 2 * HPAD], F16)
            C = spool.tile([128, PIX], F16)
            BI = spool.tile([128, 3 * T], F32)
            wsb = [None] * T
            wsb[0] = wpool.tile([128, 1280], F16, name="wsb0")
            # order: first tiles' data (h2/h3 of sample 0) before the rest
            nc.sync.dma_start(wsb[0][:, :], w_d[0, :, :])
            Q = HPAD // 4   # 8 chunks of Q cover [0, 2*HPAD)
            for q in (0, 4, 1, 5):
                sl = slice(q * Q, (q + 1) * Q)
                nc.sync.dma_start(H4[:, sl], h4_d[:, sl])
            nc.sync.dma_start(BI[:, :], bi_d[:, :])
            for q in (2, 6, 3, 7):
                sl = slice(q * Q, (q + 1) * Q)
                nc.sync.dma_start(H4[:, sl], h4_d[:, sl])
            nc.sync.dma_start(C[64:128, :], c_d[:, :])

            def emit_mms(t, s, j):
                """matmuls + psum activations for tile j (0..3) of sample s."""
                psA = psApool.tile([128, 1024], F32)
                psB = psBpool.tile([128, 1024], F32)
                for ck in (1, 0):
                    ps = psB if ck == 1 else psA
                    for nh in range(2):
                        row0 = j * ROWS_PT + nh * 8
                        p0 = _ppos(s, row0, 0)
                        for g in range(5):
                            view = GROUPS[g][2] if g < 4 else 0
                            base = view * HPAD + p0 + GROUP_OFF[g]
                            rhs = (H4[:, base:base + 8 * RS]
                                   .rearrange("p (r c) -> p r c", r=8, c=RS)
                                   [:, :, 0:W])
                            lhsT = wsb[t][:, (g * 2 + ck) * 128:
                                          (g * 2 + ck) * 128 + 128]
                            nc.tensor.matmul(
                                ps[:, nh * 512:nh * 512 + 512],
                                lhsT, rhs,
                                start=(g == 0), stop=(g == 4),
                            )
                g01 = g01pool.tile([128, 2048], F16)
                tcx = tcxpool.tile([128, 1024], F16)
                nc.scalar.activation(g01[:, 1024:2048], psB[:, :],
                                     AF.Relu, bias=BI[:, 3 * t + 1:3 * t + 2],
                                     scale=0.2)
                nc.scalar.activation(tcx[64:128, :], psB[64:128, :],
                                     AF.Tanh,
                                     bias=BI[64:128, 3 * t + 2:3 * t + 3])
                nc.scalar.activation(g01[:, 0:1024], psA[:, :],
                                     AF.Relu, bias=BI[:, 3 * t:3 * t + 1],
                                     scale=0.2)
                return g01, tcx

            def emit_min_u(g01, tcx):
                """clip + of-cross + u = i'*tC (upper half)."""
                of = ofpool.tile([128, 2048], F16)
                u = upool.tile([128, 1024], F16)
                nc.vector.tensor_scalar_min(g01[:, :], g01[:, :], 1.0)
                nc.sync.dma_start(of[64:128, :], g01[0:64, :])
                nc.vector.tensor_tensor(u[64:128, :], g01[64:128, 0:1024],
                                        tcx[64:128, :], op=ALU.mult)
                return of, u

            def emit_vc(pt, of, u):
                """c-state update for tile pt (all upper half)."""
                tsl = slice(pt * 1024, (pt + 1) * 1024)
                nc.vector.tensor_tensor(C[64:128, tsl], of[64:128, 1024:2048],
                                        C[64:128, tsl], op=ALU.mult)
                nc.vector.tensor_tensor(C[64:128, tsl], C[64:128, tsl],
                                        u[64:128, :], op=ALU.add)

            def emit_finish(s, pt, of):
                """tanh2 + h-write into H4-h2@U interior for tile pt."""
                tsl = slice(pt * 1024, (pt + 1) * 1024)
                rg = pt % TPS
                tc2 = tc2pool.tile([128, 1024], F16)
                nc.scalar.activation(tc2[64:128, :], C[64:128, tsl], AF.Tanh)
                a0 = _ppos(s, rg * ROWS_PT, 0)
                hdst = (H4[64:128, a0:a0 + ROWS_PT * RS]
                        .rearrange("p (r c) -> p r c", r=ROWS_PT, c=RS)
                        [:, :, 0:W])
                nc.vector.tensor_tensor(
                    hdst,
                    of[64:128, 0:1024].rearrange("p (r c) -> p r c",
                                                 r=ROWS_PT, c=W),
                    tc2[64:128, :].rearrange("p (r c) -> p r c",
                                             r=ROWS_PT, c=W),
                    op=ALU.mult)

            def emit_pair_dmas(t, s, jhi):
                """shifted copies + HBM out for tiles (jhi-1, jhi)."""
                pt = s * TPS + jhi
                lo = GUARD + s * SBLK + ((jhi - 1) * ROWS_PT + 1) * RS
                hi = lo + 2 * ROWS_PT * RS
                if t < T - 1:
                    # cross-partition shifted lowers: h2@L (-RS), h3@L (-1)
                    nc.sync.dma_start(H4[0:64, lo - RS:hi - RS],
                                      H4[64:128, lo:hi])
                    nc.sync.dma_start(H4[0:64, HPAD + lo - 1:HPAD + hi - 1],
                                      H4[64:128, lo:hi])
                    # same-partition canonical copy: h3@U (Pool engine)
                    nc.gpsimd.tensor_copy(H4[64:128, HPAD + lo:HPAD + hi],
                                          H4[64:128, lo:hi])
                a0p = _ppos(s, (jhi - 1) * ROWS_PT, 0)
                src = (H4[64:128, a0p:a0p + 2 * ROWS_PT * RS]
                       .rearrange("p (r c) -> p r c", r=2 * ROWS_PT, c=RS)
                       [:, :, 0:W])
                osl = slice((pt - 1) * 1024, (pt + 1) * 1024)
                nc.sync.dma_start(
                    o_d[t, :, osl].rearrange("f (r c) -> f r c",
                                             r=2 * ROWS_PT, c=W),
                    src)

            # global 3-deep block pipeline over half-steps
            live = {}   # block index -> stage state
            def block_ts(b):
                k, j = divmod(b, TPS)
                return k // 2, k % 2, j

            for b in range(NBLK + 2):
                if b < NBLK:
                    t, s, j = block_ts(b)
                    if j == 0 and s == 0 and t + 1 < T:
                        wsb[t + 1] = wpool.tile([128, 1280], F16,
                                                name=f"wsb{t + 1}")
                        nc.sync.dma_start(wsb[t + 1][:, :], w_d[t + 1, :, :])
                    g01, tcx = emit_mms(t, s, j)
                    of, u = emit_min_u(g01, tcx)
                    live[b] = (of, u)
                if b >= 1 and b - 1 < NBLK:
                    t1, s1, j1 = block_ts(b - 1)
                    of1, u1 = live[b - 1]
                    emit_vc(s1 * TPS + j1, of1, u1)
                if b >= 2:
                    tf, sf, jf = block_ts(b - 2)
                    off, _ = live.pop(b - 2)
                    emit_finish(sf, sf * TPS + jf, off)
                    if jf % 2 == 1:
                        emit_pair_dmas(tf, sf, jf)

    nc.compile()
    return nc


_CACHED_NC = None


def _get_nc():
    global _CACHED_NC
    if _CACHED_NC is None:
        _CACHED_NC = build_program()
    return _CACHED_NC


def _host_pack(inputs, h0, c0, kernels, rec_kernels, bias):
    f16 = mybir.dt.np(F16)
    Wf = (kernels + rec_kernels).astype(np.float32)  # [T,3,3,64,256]
    wblob = np.zeros((T, 128, 1280), np.float32)
    for g in range(4):
        ta, tb, _ = GROUPS[g]
        for ck, (glo, gup) in enumerate(CK_GATES):
            col = (g * 2 + ck) * 128
            wblob[:, 64:128, col:col + 64] = Wf[:, ta[0], ta[1], :, glo:glo + 64]
            wblob[:, 64:128, col + 64:col + 128] = Wf[:, ta[0], ta[1], :, gup:gup + 64]
            wblob[:, 0:64, col:col + 64] = Wf[:, tb[0], tb[1], :, glo:glo + 64]
            wblob[:, 0:64, col + 64:col + 128] = Wf[:, tb[0], tb[1], :, gup:gup + 64]
    for ck, (glo, gup) in enumerate(CK_GATES):
        col = (4 * 2 + ck) * 128
        wblob[:, 0:64, col:col + 64] = Wf[:, 2, 2, :, glo:glo + 64]
        wblob[:, 0:64, col + 64:col + 128] = Wf[:, 2, 2, :, gup:gup + 64]
    wblob = np.ascontiguousarray(wblob.astype(f16))

    bz = bias.astype(np.float32)  # [T, 256]
    biases = np.zeros((128, 3 * T), np.float32)
    for t in range(T):
        biases[0:64, 3 * t] = 0.2 * bz[t, 192:256] + 0.5      # o (ck0 @L)
        biases[64:128, 3 * t] = 0.2 * bz[t, 0:64] + 0.5       # i (ck0 @U)
        biases[0:64, 3 * t + 1] = 0.2 * bz[t, 64:128] + 0.5   # f (ck1 @L)
        biases[64:128, 3 * t + 2] = bz[t, 128:192]            # c tanh bias
    biases = np.ascontiguousarray(biases)

    in_maps = []
    for core in range(NCORES):
        b0 = core * BLOC
        hp = np.zeros((64, HPAD), np.float32)
        cpx = np.zeros((64, PIX), np.float32)
        for s in range(BLOC):
            hsamp = np.transpose(h0[b0 + s], (2, 0, 1)).reshape(64, H * W)
            csamp = np.transpose(c0[b0 + s], (2, 0, 1)).reshape(64, H * W)
            for y in range(H):
                p = _ppos(s, y, 0)
                hp[:, p:p + W] = hsamp[:, y * W:(y + 1) * W]
            cpx[:, s * H * W:(s + 1) * H * W] = csamp
        h2i = np.zeros((128, HPAD), np.float32)
        h3i = np.zeros((128, HPAD), np.float32)
        h2i[64:128] = hp
        h3i[64:128] = hp
        h2i[0:64, :HPAD - RS] = hp[:, RS:]
        h3i[0:64, :HPAD - 1] = hp[:, 1:]
        h4 = np.concatenate([h2i, h3i], axis=1)
        in_maps.append({
            "h4_init": np.ascontiguousarray(h4.astype(f16)),
            "c_init": np.ascontiguousarray(cpx.astype(f16)),
            "wblob": wblob,
            "biases": biases,
        })
    return in_maps


def kernel(inputs, h0, c0, kernels, rec_kernels, bias):
    global LAST_EXEC_NS, LAST_RESULTS
    h0 = np.asarray(h0, np.float32)
    c0 = np.asarray(c0, np.float32)
    kernels = np.asarray(kernels, np.float32)
    rec_kernels = np.asarray(rec_kernels, np.float32)
    bias = np.asarray(bias, np.float32)

    nc = _get_nc()
    in_maps = _host_pack(inputs, h0, c0, kernels, rec_kernels, bias)
    import os
    trace = bool(int(os.environ.get("K_TRACE", "0")))
    res = run_bass_kernel_spmd(nc, in_maps, core_ids=list(range(NCORES)),
                               trace=trace)
    LAST_RESULTS = res
    LAST_EXEC_NS = res.exec_time_ns
    if LAST_EXEC_NS is None:
        try:
            from concourse.timeline_sim import TimelineSim
            LAST_EXEC_NS = int(TimelineSim(nc, no_exec=True).simulate())
        except Exception:
            pass

    out = np.empty((B, T, H, W, F), np.float32)
    for core in range(NCORES):
        o = res.results[core]["out"].astype(np.float32).reshape(T, F, BLOC, H, W)
        out[core * BLOC:(core + 1) * BLOC] = np.transpose(o, (2, 0, 3, 4, 1))
    return out
